# revision 1
# baseline (speedup 1.0000x reference)
"""Trainium2 Bass kernel for the 4-kernel MMD permutation test (nn_DUAL_78237124264373).

Math (per core, 25 of the 200 permutations; everything else replicated):
  Z = [X; Y] (768 x 64), d2[r,c] = ||Z_r - Z_c||^2 built on the PE as a single
  rank-66 matmul  d2 = L^T R  with L = [Zt; sq; 1], R = [-2 Zt; 1; sq].
  K0_k = f_k(d2) (symmetric kernel matrix, no diag zeroing).
  With a_p the X-half indicator of permutation p and the zeroed-K statistics
  expressed through symmetric-K0 quantities plus corrections through
  e_j = K0[j, 384+j] (the zeroed stripe), every U_b entry reduces to
     U_b = kap*(q0 - arow0) + W_corr @ e_k + (2/c2)*t + C_k
  where q0 = a K0 a, arow0 = a K0 1 come from one matmul M0 = A_aug K0,
  t is the per-permutation paired-sample sum computed from host-arranged
  Z-row pairs (sentinel rows handle zeroed-stripe pairs), and W_corr folds
  the three correction coefficients into one host-built matrix.

Layout: the four kernels are column-tiled onto PE col-groups, so all
per-permutation statistics live at partition 32*k + p (kernel k, perm p) and
the DVE reductions run once over 128 partitions instead of 4x over 27.
"""

import os
import sys

import numpy as np

if "/opt/trn_rl_repo" not in sys.path:
    sys.path.insert(0, "/opt/trn_rl_repo")

import concourse.bacc as bacc
import concourse.bass as bass
import concourse.mybir as mybir
import concourse.tile as tile
from concourse import bass_utils

N = 384
NM = 768
D = 64
NPER = 200
NC = 8
PPC = NPER // NC  # 25
C1 = float(N * (N - 1))
C2 = float(N * N)
KAP = np.float32(2.0 / C1 + 2.0 / C2)
CB1 = np.float32(1.0 / C1 + 2.0 / C2)
CB2 = np.float32(1.0 / C1)
TCO = np.float32(2.0 / C2)
IC1 = np.float32(1.0 / C1)
IC2 = np.float32(1.0 / C2)
KERNELS = ("gaussian", "laplacian", "gaussian", "laplacian")

F32 = mybir.dt.float32
F32R = mybir.dt.float32r
BF16 = mybir.dt.bfloat16
AF = mybir.ActivationFunctionType
ALU = mybir.AluOpType


def _build():
    nc = bacc.Bacc("TRN2", target_bir_lowering=False, debug=False)
    with tile.TileContext(nc) as tc:
        with tc.tile_pool(name="dram", bufs=1, space="DRAM") as dram, \
             tc.tile_pool(name="io", bufs=1) as io, \
             tc.tile_pool(name="big", bufs=1) as big, \
             tc.tile_pool(name="kpool", bufs=4) as kpool, \
             tc.tile_pool(name="scr", bufs=2) as scr, \
             tc.tile_pool(name="sml", bufs=1) as sml:

            def din(name, shape, dt=F32):
                return dram.tile(shape, dt, kind="ExternalInput", name=name,
                                 uniquify=False)

            # One fused input tensor (single DMA); column layout below.
            W_IN = 2002
            bigin_d = din("bigin", [128, W_IN])
            zp_d = din("zp", [128, 9984 + 192], BF16)  # pair rows + bf16 atp
            out_d = dram.tile([4, 1 + PPC], F32, kind="ExternalOutput",
                              name="out", uniquify=False)

            # ---- phase 0: input DMAs (Zt block first: it gates the PE) ----
            bigin = io.tile([128, W_IN], F32, name="bigin_sb")
            nc.sync.dma_start(out=bigin[:, 0:NM], in_=bigin_d[:, 0:NM])
            nc.sync.dma_start(out=bigin[:, NM:], in_=bigin_d[:, NM:])
            Lbig = bigin[0:D + 1, 0:NM]          # [Zt rows 0-63; ones row 64]
            astk = bigin[:, NM:2 * NM]           # A_aug rows at 32k+i
            atp = bigin[:, 1536:1536 + 192]      # A_aug^T chunks (32-padded)
            wct = bigin[:, 1728:1728 + 96]       # W_corr^T chunks (32-padded)
            fold = bigin[0:75, 1824:1824 + 32]   # 3->1 fold (32-padded)
            ident = bigin[:, 1856:1856 + 128]
            aux = bigin[:, 1984:1994]
            aux4 = bigin[0:1, 1994:2002]
            zpf = io.tile([128, 78 * 128 + 192], BF16, name="zp_sb")
            nc.sync.dma_start(out=zpf[:], in_=zp_d[:])
            zp = zpf[:, 0:9984].rearrange("p (b d) -> p b d", d=128)
            atpb = zpf[:, 9984:9984 + 192]       # A_aug^T chunks in bf16

            ones = io.tile([128, 1], F32, name="ones_sb")
            nc.vector.memset(ones[:], 1.0)

            R_all = io.tile([D + 1, NM], F32, name="R_all")
            # cols 0:4608 = the 6 row-tiles of d2; cols 4608:4686 = the 78
            # pair-distance columns, so ONE wide sqrt covers both.
            d2sb = big.tile([128, 6 * NM + 78], F32, name="d2sb")
            dist = big.tile([128, 6 * NM + 78], F32, name="dist_sb")
            M0sb = big.tile([128, NM], F32, name="M0sb")

            with tc.tile_pool(name="psA", bufs=3, space="PSUM") as psA:
                # ---- phase 1: sq = rowsums of Zt^2, landed at psum
                # partitions 0 (for the sq_col transposes) and 64 (for the
                # R matrix row) via col-tiling ----
                zt2 = scr.tile([D, NM], F32, name="zt2", tag="zt2", bufs=1)
                nc.vector.tensor_tensor(out=zt2[:], in0=Lbig[0:D, :],
                                        in1=Lbig[0:D, :], op=ALU.mult)
                ps_sq = psA.tile([128, NM], F32, name="ps_sq", tag="d2")
                for s in (slice(0, 512), slice(512, NM)):
                    nc.tensor.matmul(ps_sq[0:1, s], ones[0:D, 0:1], zt2[:, s],
                                     start=True, stop=True,
                                     skip_group_check=True)
                    nc.tensor.matmul(ps_sq[D:D + 1, s], ones[0:D, 0:1],
                                     zt2[:, s], start=True, stop=True,
                                     tile_position=(0, D),
                                     skip_group_check=True)
                sqrow = sml.tile([1, NM], F32, name="sqrow")
                nc.vector.tensor_copy(sqrow[:], ps_sq[0:1, :])
                # R = [-2 Zt; sq]; row 64 copies within partition 64.
                nc.vector.tensor_scalar_mul(R_all[0:D, :], Lbig[0:D, :], -2.0)
                nc.vector.tensor_copy(R_all[D:D + 1, :], ps_sq[D:D + 1, :])
                # sq as columns (for the relu bias): 6 tiny PE transposes
                ps_sqc = psA.tile([128, 8], F32, name="ps_sqc", tag="sqc",
                                  bufs=1)
                for r in range(6):
                    nc.tensor.matmul(ps_sqc[:, r:r + 1],
                                     sqrow[0:1, 128 * r:128 * (r + 1)],
                                     ones[0:1, 0:1], is_transpose=True,
                                     start=True, stop=True,
                                     skip_group_check=True)
                sqc = sml.tile([128, 8], F32, name="sqc")
                nc.vector.tensor_copy(sqc[:, 0:6], ps_sqc[:, 0:6])
                # per-gaussian fused bias: (sq[r] + 1e-12) * scale_k
                sqsc = {}
                for k in (0, 2):
                    t = sml.tile([128, 8], F32, name=f"sqsc{k}")
                    nc.vector.tensor_scalar(
                        out=t[:, 0:6], in0=sqc[:, 0:6],
                        scalar1=aux[:, 2 * k:2 * k + 1],
                        scalar2=aux[:, 2 * k + 1:2 * k + 2],
                        op0=ALU.mult, op1=ALU.add)
                    sqsc[k] = t

                kts = [kpool.tile([128, 6 * NM], BF16, name=f"kt{k}",
                                  tag="kt") for k in range(4)]

                # ---- phase 2: psum = -2 Z Z^T + sq[c].  Per row-tile: the
                # DVE adds sq[r] and clamps into d2sb while the two gaussian
                # kernels exp straight out of PSUM (exp of the tiny negative
                # diagonal values is harmless). The DVE also squeezes the
                # bf16 pair-distance pieces into its matmul-wait gaps. ----
                pdiff = sml.tile([128, 78, 64], BF16, name="pdiff")
                pprod = sml.tile([128, 78, 64], BF16, name="pprod")
                for r in range(6):
                    ps_d2 = psA.tile([128, NM], F32, name=f"ps_d2_{r}",
                                     tag="d2")
                    lhs = Lbig[:, 128 * r:128 * (r + 1)]
                    nc.tensor.matmul(ps_d2[:, 0:512], lhs, R_all[:, 0:512],
                                     start=True, stop=True)
                    nc.tensor.matmul(ps_d2[:, 512:NM], lhs, R_all[:, 512:NM],
                                     start=True, stop=True)
                    sl = slice(NM * r, NM * (r + 1))
                    nc.vector.tensor_scalar(
                        out=d2sb[:, sl], in0=ps_d2[:],
                        scalar1=sqc[:, r:r + 1], scalar2=0.0,
                        op0=ALU.add, op1=ALU.max)
                    for k in (0, 2):
                        nc.scalar.activation(kts[k][:, sl], ps_d2[:], AF.Exp,
                                             scale=aux[:, 2 * k:2 * k + 1],
                                             bias=sqsc[k][:, r:r + 1])
                    j = r if r < 3 else r - 3
                    js = slice(26 * j, 26 * (j + 1))
                    if r < 3:
                        nc.vector.tensor_tensor(out=pdiff[:, js, :],
                                                in0=zp[:, js, 0:64],
                                                in1=zp[:, js, 64:128],
                                                op=ALU.subtract)
                    else:
                        nc.vector.tensor_tensor(out=pprod[:, js, :],
                                                in0=pdiff[:, js, :],
                                                in1=pdiff[:, js, :],
                                                op=ALU.mult)
                for j in range(3):
                    js = slice(26 * j, 26 * (j + 1))
                    nc.vector.tensor_reduce(
                        d2sb[:, 6 * NM + 26 * j:6 * NM + 26 * (j + 1)],
                        pprod[:, js, :], axis=mybir.AxisListType.X,
                        op=ALU.add)

            # ---- phase 4: dist = sqrt(d2 + 1e-12), pair cols included ----
            nc.scalar.activation(dist[:], d2sb[:], AF.Sqrt, bias=aux[:, 8:9])
            distp = dist[:, 6 * NM:6 * NM + 78]

            arow = sml.tile([128, 1], F32, name="arow")
            colA = sml.tile([128, 1], F32, name="colA")
            q0c = sml.tile([128, 1], F32, name="q0c")

            with tc.tile_pool(name="psB", bufs=1, space="PSUM") as psB, \
                 tc.tile_pool(name="psC", bufs=1, space="PSUM") as psC:
                # ---- phase 5: laplacian K tiles; M0 = A_aug K0 col-tiled so
                # kernel k's rows land at partitions 32k+i ----
                ps_m = psB.tile([128, NM], F32, name="ps_m")
                for k in (1, 3):
                    for h in range(2):
                        hs = slice(3 * NM * h, 3 * NM * (h + 1))
                        nc.scalar.activation(kts[k][:, hs], dist[:, hs],
                                             AF.Exp,
                                             scale=aux[:, 2 * k:2 * k + 1],
                                             bias=aux[:, 2 * k + 1:2 * k + 2])
                for c in range(6):
                    lhs = atpb[:, 32 * c:32 * (c + 1)]
                    for k in range(4):
                        pr = slice(32 * k, 32 * k + 32)
                        nc.tensor.matmul(ps_m[pr, 0:512], lhs,
                                         kts[k][:, NM * c:NM * c + 512],
                                         start=(c == 0), stop=(c == 5),
                                         tile_position=(0, 32 * k),
                                         skip_group_check=True)
                        nc.tensor.matmul(ps_m[pr, 512:NM], lhs,
                                         kts[k][:, NM * c + 512:NM * (c + 1)],
                                         start=(c == 0), stop=(c == 5),
                                         tile_position=(0, 32 * k),
                                         skip_group_check=True)
                # row stats: copy+rowsum fused, first-half sum, masked q0
                nc.vector.tensor_scalar(
                    out=M0sb[:], in0=ps_m[:], scalar1=1.0, scalar2=0.0,
                    op0=ALU.mult, op1=ALU.add, accum_out=arow[:])
                sA = scr.tile([128, N], F32, name="sA", tag="sA")
                nc.vector.tensor_scalar(
                    out=sA[:], in0=M0sb[:, 0:N], scalar1=1.0, scalar2=0.0,
                    op0=ALU.mult, op1=ALU.add, accum_out=colA[:])
                sB = scr.tile([128, NM], F32, name="sB", tag="sB")
                nc.vector.tensor_tensor(out=sB[:], in0=M0sb[:], in1=astk[:],
                                        op=ALU.mult)
                nc.vector.tensor_reduce(q0c[:], sB[:],
                                        axis=mybir.AxisListType.X, op=ALU.add)

                # ---- pair-term exps (Exp table is already loaded) ----
                # t_k via column-sum matmul then a fold matmul into
                # partitions 32k+p
                d2p = d2sb[:, 6 * NM:6 * NM + 78]
                ps_t = psC.tile([75, 4], F32, name="ps_t", tag="sm", bufs=3)
                expks = []
                for k, kern in enumerate(KERNELS):
                    psrc = d2p if kern == "gaussian" else distp
                    expk = scr.tile([128, 78], F32, name=f"expk{k}",
                                    tag="expk", bufs=4)
                    nc.scalar.activation(expk[:], psrc, AF.Exp,
                                         scale=aux[:, 2 * k:2 * k + 1],
                                         bias=aux[:, 2 * k + 1:2 * k + 2])
                    expks.append(expk)
                    nc.tensor.matmul(ps_t[:, k:k + 1], expk[:, 0:75],
                                     ones[:, 0:1], start=True, stop=True)
                t75s = sml.tile([75, 4], F32, name="t75s")
                nc.vector.tensor_copy(t75s[:], ps_t[:])
                ps_tc = psC.tile([128, 1], F32, name="ps_tc", tag="sm", bufs=3)
                for k in range(4):
                    nc.tensor.matmul(ps_tc[32 * k:32 * k + 32, 0:1], fold[:],
                                     t75s[:, k:k + 1], start=True, stop=True,
                                     tile_position=(0, 32 * k),
                                     skip_group_check=True)
                tcol = sml.tile([128, 1], F32, name="tcol")
                nc.vector.tensor_scalar_mul(tcol[:], ps_tc[:], float(TCO))

                # ---- phase 6: corrections (col-tiled) and stripe sums ----
                ps_corr = psC.tile([128, 1], F32, name="ps_corr", tag="sm",
                                   bufs=3)
                for c in range(3):
                    for k in range(4):
                        nc.tensor.matmul(
                            ps_corr[32 * k:32 * k + 32, 0:1],
                            wct[:, 32 * c:32 * (c + 1)],
                            expks[k][:, 75 + c:76 + c],
                            start=(c == 0), stop=(c == 2),
                            tile_position=(0, 32 * k),
                            skip_group_check=True)
                sesum = sml.tile([3, 4], F32, name="sesum")
                for k in range(4):
                    ps_sek = psC.tile([3, 1], F32, name=f"ps_se{k}", tag="se",
                                      bufs=2)
                    nc.tensor.matmul(ps_sek[:], expks[k][:, 75:78],
                                     ones[:, 0:1], start=True, stop=True)
                    nc.vector.tensor_copy(sesum[:, k:k + 1], ps_sek[:])

                # ---- phase 7: U_b assembly in the stacked [128,1] layout ----
                colB = sml.tile([128, 1], F32, name="colB")
                nc.vector.tensor_tensor(out=colB[:], in0=arow[:], in1=colA[:],
                                        op=ALU.subtract)
                ubv = sml.tile([128, 1], F32, name="ubv")
                nc.vector.tensor_tensor(out=ubv[:], in0=q0c[:], in1=arow[:],
                                        op=ALU.subtract)
                nc.vector.tensor_scalar_mul(ubv[:], ubv[:], float(KAP))
                nc.vector.tensor_tensor(out=ubv[:], in0=ubv[:], in1=ps_corr[:],
                                        op=ALU.add)
                nc.vector.tensor_tensor(out=ubv[:], in0=ubv[:], in1=tcol[:],
                                        op=ALU.add)
                # ---- phase 8: fold everything into one partition-0 row ----
                # frow: [0:128)=ub, [128:256)=colA^T, [256:384)=colB^T,
                # [384:396)=sesum
                frow = sml.tile([1, 396], F32, name="frow")
                nc.sync.dma_start(out=frow[0:1, 0:128], in_=ubv[:])
                nc.sync.dma_start(out=frow[0:1, 128:256], in_=colA[:])
                nc.sync.dma_start(out=frow[0:1, 256:384], in_=colB[:])
                nc.sync.dma_start(out=frow[0:1, 384:396], in_=sesum[:])

                def fr(base, step=32, count=4):
                    ap = frow[0:1, base:base + 1]
                    return bass.AP(ap.tensor, ap.offset,
                                   [ap.ap[0], [step, count]])

                XXv = fr(128 + 25)
                YXv = fr(128 + 26)
                XY0v = fr(256 + 25)
                YYv = fr(256 + 26)
                # se_k = sum_c sesum[4c+k]
                sev = sml.tile([1, 4], F32, name="sev")
                nc.vector.tensor_reduce(
                    sev[:],
                    frow[0:1, 384:396].rearrange("o (c k) -> o k c", k=4),
                    axis=mybir.AxisListType.X, op=ALU.add)
                s0t = sml.tile([1, 4], F32, name="s0t")
                nc.vector.tensor_tensor(out=s0t[:], in0=XXv, in1=YXv,
                                        op=ALU.add)
                nc.vector.tensor_tensor(out=s0t[:], in0=s0t[:], in1=XY0v,
                                        op=ALU.add)
                nc.vector.tensor_tensor(out=s0t[:], in0=s0t[:], in1=YYv,
                                        op=ALU.add)
                ck = sml.tile([1, 4], F32, name="ck")
                nc.vector.tensor_tensor(out=ck[:], in0=s0t[:], in1=sev[:],
                                        op=ALU.subtract)
                nc.vector.tensor_tensor(out=ck[:], in0=ck[:],
                                        in1=aux4[0:1, 0:4], op=ALU.subtract)
                nc.vector.tensor_scalar_mul(ck[:], ck[:], float(IC1))
                u1 = sml.tile([1, 4], F32, name="u1")
                nc.vector.tensor_tensor(out=u1[:], in0=XXv, in1=YYv,
                                        op=ALU.add)
                nc.vector.tensor_tensor(out=u1[:], in0=u1[:],
                                        in1=aux4[0:1, 0:4], op=ALU.subtract)
                nc.vector.tensor_scalar_mul(u1[:], u1[:], float(IC1))
                u2 = sml.tile([1, 4], F32, name="u2")
                nc.vector.tensor_tensor(out=u2[:], in0=XY0v, in1=sev[:],
                                        op=ALU.subtract)
                nc.vector.tensor_scalar_mul(u2[:], u2[:], float(2.0 * IC2))

                # ---- phase 9: contiguous U row + U_b block, two out DMAs ----
                uF = sml.tile([1, 4], F32, name="uF")
                nc.vector.tensor_tensor(out=uF[:], in0=u1[:], in1=u2[:],
                                        op=ALU.subtract)
                ubc = sml.tile([1, 4 * PPC], F32, name="ubc")
                ub_src = frow[0:1, 0:128].rearrange("o (k p) -> o k p", p=32)
                ckap = ck[0:1, 0:4]
                ck_b = bass.AP(ckap.tensor, ckap.offset,
                               [ckap.ap[0], [1, 4], [0, PPC]])
                nc.vector.tensor_tensor(
                    out=ubc[0:1, :].rearrange("o (k p) -> o k p", p=PPC),
                    in0=ub_src[0:1, :, 0:PPC], in1=ck_b, op=ALU.add)
                nc.sync.dma_start(
                    out=out_d[:, 0:1],
                    in_=uF[0:1, :].rearrange("o (k w) -> o k w", w=1))
                nc.sync.dma_start(
                    out=out_d[:, 1:1 + PPC],
                    in_=ubc[0:1, :].rearrange("o (k p) -> o k p", p=PPC))

    nc.compile()
    return nc


def _host_prep(X, Y, bandwidths, perms):
    X = np.ascontiguousarray(X, np.float32)
    Y = np.ascontiguousarray(Y, np.float32)
    perms = np.ascontiguousarray(perms, np.int32)
    Zt = np.zeros((D + 1, NM), np.float32)  # rows 0-63 Zt, row 64 ones
    Zt[0:D] = np.concatenate([X, Y], 0).T
    Zt[D] = 1.0
    b = np.asarray(bandwidths, np.float64)
    gs = (-1.0 / (b * b)).astype(np.float32)
    gb = (gs.astype(np.float64) * 1e-12).astype(np.float32)
    ls = (-1.0 / b).astype(np.float32)
    aux = np.zeros((128, 10), np.float32)
    aux[:, 8] = 1e-12
    d0c = np.zeros(4, np.float64)
    for k, kern in enumerate(KERNELS):
        if kern == "gaussian":
            aux[:, 2 * k] = gs[k]
            aux[:, 2 * k + 1] = gb[k]
            d0c[k] = np.exp(-1e-12 / (b[k] * b[k]))
        else:
            aux[:, 2 * k] = ls[k]
            aux[:, 2 * k + 1] = 0.0
            d0c[k] = np.exp(-np.sqrt(1e-12) / b[k])
    aux4 = np.zeros((1, 8), np.float32)
    aux4[0, 0:4] = (768.0 * d0c).astype(np.float32)
    ident = np.eye(128, dtype=np.float32)
    foldm = np.zeros((75, 32), np.float32)
    foldm[:, :PPC] = (np.arange(75)[:, None] // 3 ==
                      np.arange(PPC)[None, :])

    maps = []
    for cid in range(NC):
        pm = perms[cid * PPC:(cid + 1) * PPC]
        A = np.zeros((27, NM), np.float32)
        A[np.arange(PPC)[:, None], pm[:, :N]] = 1
        A[25, :N] = 1
        A[26, N:] = 1
        astk = np.zeros((128, NM), np.float32)
        for k in range(4):
            astk[32 * k:32 * k + 27] = A
        atp = np.zeros((128, 6 * 32), np.float32)
        for c in range(6):
            atp[:, 32 * c:32 * c + 27] = A[:, 128 * c:128 * (c + 1)].T
        A1 = A[:PPC, :N]
        A2 = A[:PPC, N:]
        Wc = (-KAP * (A1 * A2) + CB1 * A1 + CB2 * A2).astype(np.float32)
        wct = np.zeros((128, 3 * 32), np.float32)
        for c in range(3):
            wct[:, 32 * c:32 * c + PPC] = Wc[:, 128 * c:128 * (c + 1)].T
        pX = pm[:, :N].astype(np.int64).ravel()
        pY = pm[:, N:].astype(np.int64).ravel()
        # Pair-arranged Z rows: [zx | zy] per pair; stripe pairs (pY==pX+384)
        # get a sentinel row with huge distance so f_k -> 0 (matches the
        # zeroed K stripe). Rows 9600..9983 are the stripe-diagonal pairs
        # (they produce the e_k correction vectors).
        Zf = np.concatenate([X, Y], 0)
        zx = Zf[pX]
        zy = Zf[pY]
        stripe = pY == pX + N
        zx[stripe] = 0.0
        zy[stripe] = 0.0
        zx[stripe, 0] = 1e6  # d2=1e12: exp(-1e12/b^2)=exp(-1e6/b)=0
        j = np.arange(N)
        zp = np.concatenate([
            np.concatenate([zx, zy], 1),
            np.concatenate([Zf[j], Zf[N + j]], 1),
        ], 0)
        import ml_dtypes
        zp = zp.reshape(78, 128, 128).transpose(1, 0, 2).reshape(128, 9984)
        zp = np.concatenate([zp, atp], 1).astype(ml_dtypes.bfloat16)
        bigin = np.zeros((128, 2002), np.float32)
        bigin[0:D + 1, 0:NM] = Zt
        bigin[:, NM:2 * NM] = astk
        bigin[:, 1536:1536 + 192] = atp
        bigin[:, 1728:1728 + 96] = wct
        bigin[0:75, 1824:1824 + 32] = foldm
        bigin[:, 1856:1856 + 128] = ident
        bigin[:, 1984:1994] = aux
        bigin[0:1, 1994:2002] = aux4
        maps.append(dict(bigin=bigin, zp=zp))
    return maps


_NC_CACHE = None


def _get_nc():
    global _NC_CACHE
    if _NC_CACHE is None:
        _NC_CACHE = _build()
    return _NC_CACHE


def kernel(X, Y, bandwidths, perms):
    nc = _get_nc()
    in_maps = _host_prep(X, Y, bandwidths, perms)
    res = bass_utils.run_bass_kernel_spmd(nc, in_maps, list(range(NC)))
    full = np.zeros((4, 1 + NPER), np.float32)
    full[:, 0] = res.results[0]["out"][:, 0]
    for cid in range(NC):
        full[:, 1 + cid * PPC:1 + (cid + 1) * PPC] = \
            res.results[cid]["out"][:, 1:]
    return full



# revision 22
# speedup vs baseline: 1.3587x; 1.3587x over previous
"""Trainium2 Bass kernel for the 4-kernel MMD permutation test (nn_DUAL_78237124264373).

Sharding: 8 cores = 2 kernel-pairs x 4 permutation quarters. Core c<4 computes
kernels (0,1) [gaussian, laplacian] for perms [50*(c%4), 50*(c%4)+50); core
c>=4 the same for kernels (2,3). The host merges the [2, 1+50] per-core
outputs, so each core only ever evaluates TWO kernel matrices and the
activation-table sequence is exactly EXP (pre-warmed) -> SQRT -> EXP: the
swap points are pinned with zero-valued bias/scale tokens that data-depend
on the previous block's last op, so the Tile scheduler cannot interleave.

Per-core pipeline (slot a = gaussian, slot b = laplacian):
  d2 = L^T R on the PE in f32r (L = [Zt; 1], R = [-2 Zt; sq + B]), 12 PSUM
  pieces (6 row tiles x 512+256) in a 4-deep PSUM pool. Slot-a K = exp(ga*d2)
  straight out of PSUM with its M0 = A_aug K chunk matmul right behind, while
  the DVE lands clamped d2 in SBUF for the sqrt block. After the swap back to
  EXP, slot-b K = exp(lb*dist) runs chunk-by-chunk with M0 trailing. Each
  slot's row stats (aKa, aK1, colA), U_b vector and PE transpose into the
  partition-0 assembly row run as soon as that slot's M0 completes, so slot
  a's tail hides under the SQRT/slot-b window. U_b = KAP*(aKa - aK1) +
  W_corr @ e + (2/c2)*t + ck, with pair sums t reduced from host-gathered
  16-wide partial squares and e the K0[j, 384+j] stripe (3 extra pair
  blocks). The final scalar assembly (U, ck) happens on partition 0.
"""

import sys

import numpy as np

if "/opt/trn_rl_repo" not in sys.path:
    sys.path.insert(0, "/opt/trn_rl_repo")

import ml_dtypes

import concourse.bacc as bacc
import concourse.bass as bass
import concourse.mybir as mybir
import concourse.tile as tile
from concourse import bass_utils

N = 384
NM = 768
D = 64
NPER = 200
NC = 8
PPC = 50                      # perms per core
ROWS = PPC + 2                # + X-identity + Y-identity rows
NBLK = 3 * PPC + 3            # pair blocks of 128: 50 perms x 3 + stripe x 3
BIAS = 1e-3                   # keeps d2 > 0 under f32r rounding (see d0c)
C1 = float(N * (N - 1))
C2 = float(N * N)
KAP = np.float32(2.0 / C1 + 2.0 / C2)
CB1 = np.float32(1.0 / C1 + 2.0 / C2)
CB2 = np.float32(1.0 / C1)
TCO = np.float32(2.0 / C2)
IC1 = np.float32(1.0 / C1)
IC2 = np.float32(1.0 / C2)

F32 = mybir.dt.float32
F32R = mybir.dt.float32r
BF16 = mybir.dt.bfloat16
AF = mybir.ActivationFunctionType
ALU = mybir.AluOpType


def _build():
    nc = bacc.Bacc("TRN2", target_bir_lowering=False, debug=False)
    with tile.TileContext(nc) as tc:
        with tc.tile_pool(name="dram", bufs=1, space="DRAM") as dram, \
             tc.tile_pool(name="io", bufs=1) as io, \
             tc.tile_pool(name="big", bufs=1) as big, \
             tc.tile_pool(name="scr", bufs=1) as scr, \
             tc.tile_pool(name="sml", bufs=1) as sml:

            def din(name, shape, dt=F32):
                return dram.tile(shape, dt, kind="ExternalInput", name=name,
                                 uniquify=False)

            zlr_d = din("zlr", [D + 1, 2 * NM], F32R)
            psq_d = din("psq", [128, NBLK * 4], BF16)
            bfp_d = din("bfp", [128, NM + 6 * D + 3 * D], BF16)
            fsp_d = din("fsp", [128, 32], F32)
            idm_d = din("idm", [128, 64], F32)
            out_d = dram.tile([2, 1 + PPC], F32, kind="ExternalOutput",
                              name="out", uniquify=False)

            # ---- input DMAs; zlr is [R | L] so the first piece lands first
            zlr = io.tile([D + 1, 2 * NM], F32R, name="zlr_sb")
            nc.sync.dma_start(out=zlr[:, 0:896], in_=zlr_d[:, 0:896])
            nc.sync.dma_start(out=zlr[:, 896:], in_=zlr_d[:, 896:])
            psq = io.tile([128, NBLK * 4], BF16, name="psq_sb")
            nc.sync.dma_start(out=psq[:], in_=psq_d[:])
            bfp = io.tile([128, NM + 6 * D + 3 * D], BF16, name="bfp_sb")
            nc.sync.dma_start(out=bfp[:], in_=bfp_d[:])
            fsp = io.tile([128, 32], F32, name="fsp_sb")
            nc.sync.dma_start(out=fsp[:], in_=fsp_d[:])
            idm = io.tile([128, 64], F32, name="idm_sb")
            nc.sync.dma_start(out=idm[:], in_=idm_d[:])

            zr = zlr[:, 0:NM]
            zl = zlr[:, NM:2 * NM]
            astk = bfp[:, 0:NM]                      # A rows at 0-51 / 64-115
            atp = bfp[:, NM:NM + 6 * D]              # A^T chunks, 64-padded
            wct = bfp[:, NM + 6 * D:NM + 9 * D]      # W_corr^T chunks
            sqc = fsp[:, 0:6]                        # sq columns per row tile
            gbias = fsp[:, 6:12]                     # ga * sq per row tile
            ga = fsp[:, 12:13]
            lb = fsp[:, 13:14]
            zero = fsp[:, 14:15]
            aux4i = fsp[0:1, 16:18]                  # 768*d0c*IC1 per slot

            ones = io.tile([128, 1], F32, name="ones_sb")
            nc.vector.memset(ones[:], 1.0)
            onesb = io.tile([128, 1], BF16, name="onesb_sb")
            nc.vector.memset(onesb[:], 1.0)

            d2sb = big.tile([128, 6 * NM], F32, name="d2sb")
            dist = big.tile([128, 6 * NM], F32, name="dist_sb")
            kta = big.tile([128, 6 * NM], BF16, name="kta")
            ktb = big.tile([128, 6 * NM], BF16, name="ktb")
            M0sb = big.tile([128, NM], F32, name="M0sb")
            sA = scr.tile([128, N], F32, name="sA")
            sB = scr.tile([128, NM], F32, name="sB")
            pair2 = sml.tile([128, NBLK], F32, name="pair2")
            arow = sml.tile([128, 1], F32, name="arow")
            colA = sml.tile([128, 1], F32, name="colA")
            q0c = sml.tile([128, 1], F32, name="q0c")
            pack = sml.tile([128, 4], F32, name="pack")

            # warm the EXP activation table while DMAs are in flight
            warm = sml.tile([128, 1], F32, name="warm")
            nc.scalar.activation(warm[0:1, :], ones[0:1, :], AF.Exp,
                                 bias=0.0, scale=1.0)

            with tc.tile_pool(name="psA", bufs=4, space="PSUM") as psA, \
                 tc.tile_pool(name="psB", bufs=1, space="PSUM") as psB, \
                 tc.tile_pool(name="psC", bufs=1, space="PSUM") as psC:

                ps_m = psB.tile([128, NM], F32, name="ps_m")
                ps_t = psC.tile([128, 1], F32, name="ps_t")
                ps_corr = psC.tile([128, 1], F32, name="ps_corr")
                ps_row = ps_m[0:1, 0:512]

                # ---- pair d2: reduce the host 16-wide partial squares ----
                psq3 = psq.rearrange("p (b d) -> p b d", d=4)
                nc.vector.tensor_reduce(pair2[:], psq3[:],
                                        axis=mybir.AxisListType.X, op=ALU.add)

                # ---- d2 phase: f32r matmuls in 512-col PSUM pieces; slot-a
                # exp + M0 chunk from PSUM; DVE lands d2 in SBUF for sqrt ----
                for r in range(6):
                    lhs = zl[:, 128 * r:128 * (r + 1)]
                    for fs in (slice(0, 512), slice(512, NM)):
                        w = fs.stop - fs.start
                        ps_d2 = psA.tile([128, 512], F32, tag="d2",
                                         name=f"ps_d2_{r}_{fs.start}")
                        nc.tensor.matmul(ps_d2[:, 0:w], lhs, zr[:, fs],
                                         start=True, stop=True)
                        sl = slice(NM * r + fs.start, NM * r + fs.stop)
                        nc.scalar.activation(kta[:, sl], ps_d2[:, 0:w],
                                             AF.Exp, scale=ga,
                                             bias=gbias[:, r:r + 1])
                        nc.vector.tensor_scalar(
                            out=d2sb[:, sl], in0=ps_d2[:, 0:w],
                            scalar1=sqc[:, r:r + 1], scalar2=0.0,
                            op0=ALU.add, op1=ALU.max)
                        nc.tensor.matmul(ps_m[0:64, fs],
                                         atp[:, D * r:D * r + 64],
                                         kta[:, sl],
                                         start=(r == 0), stop=(r == 5),
                                         tile_position=(0, 0),
                                         skip_group_check=True)

                # slot-a pair exp rides the warm EXP table before the swap
                pea = sml.tile([128, NBLK], BF16, name="pea")
                nc.scalar.activation(pea[:], pair2[:], AF.Exp,
                                     bias=zero, scale=ga)
                # zb: zero bias that depends on the last EXP-block op, so
                # the scheduler cannot move the SQRT block earlier
                zb1 = sml.tile([128, 1], F32, name="zb1")
                nc.vector.tensor_scalar(
                    out=zb1[:], in0=kta[:, 6 * NM - 1:6 * NM],
                    scalar1=0.0, scalar2=0.0, op0=ALU.mult, op1=ALU.add)
                zb = sml.tile([128, 1], F32, name="zb")
                nc.vector.tensor_tensor(out=zb[:], in0=zb1[:],
                                        in1=pea[:, 0:1], op=ALU.mult)

                # ---- swap to SQRT: dist halves, then pair dist ----
                for h in range(2):
                    hs = slice(3 * NM * h, 3 * NM * (h + 1))
                    nc.scalar.activation(dist[:, hs], d2sb[:, hs], AF.Sqrt,
                                         bias=zb, scale=1.0)
                zb2 = sml.tile([128, 1], F32, name="zb2")
                nc.vector.tensor_scalar(
                    out=zb2[:], in0=dist[:, 6 * NM - 1:6 * NM],
                    scalar1=0.0, scalar2=0.0, op0=ALU.mult, op1=ALU.add)
                pdist = sml.tile([128, NBLK], F32, name="pdist")
                nc.scalar.activation(pdist[:], pair2[:], AF.Sqrt,
                                     bias=zb2, scale=1.0)
                # lbt == lb, but depends on the last SQRT-block op
                lbt = sml.tile([128, 1], F32, name="lbt")
                nc.vector.tensor_scalar(
                    out=lbt[:], in0=pdist[:, 0:1], scalar1=0.0,
                    scalar2=lb, op0=ALU.mult, op1=ALU.add)

                def slot_tail(i, pe):
                    """Pair sums, corrections, row stats, ubv and the PE
                    transpose for slot i; runs as soon as its M0 stops."""
                    pt = slice(64 * i, 64 * i + 64)
                    # t3: per-perm 3-block partial sums; group PPC holds the
                    # stripe so t[50] = sum(e) lands in ps_t for free
                    pe3 = pe.rearrange("p (g t) -> p g t", t=3)
                    t3 = sml.tile([128, 64], BF16, name=f"t3_{i}")
                    nc.vector.memset(t3[:, PPC + 1:64], 0.0)
                    with nc.allow_low_precision(reason="3-wide bf16 sum"):
                        nc.vector.tensor_reduce(t3[:, 0:PPC + 1], pe3[:],
                                                axis=mybir.AxisListType.X,
                                                op=ALU.add)
                    nc.tensor.matmul(ps_t[pt, :], t3[:], onesb[:],
                                     start=True, stop=True,
                                     tile_position=(0, 64 * i),
                                     skip_group_check=True)
                    for c in range(3):
                        nc.tensor.matmul(
                            ps_corr[pt, :], wct[:, D * c:D * c + 64],
                            pe[:, 3 * PPC + c:3 * PPC + c + 1],
                            start=(c == 0), stop=(c == 2),
                            tile_position=(0, 64 * i),
                            skip_group_check=True)
                    # row stats off this slot's half of ps_m
                    nc.vector.tensor_scalar(
                        out=M0sb[pt, :], in0=ps_m[pt, :], scalar1=1.0,
                        scalar2=0.0, op0=ALU.mult, op1=ALU.add,
                        accum_out=arow[pt, :])
                    nc.vector.tensor_scalar(
                        out=sA[pt, :], in0=M0sb[pt, 0:N], scalar1=1.0,
                        scalar2=0.0, op0=ALU.mult, op1=ALU.add,
                        accum_out=colA[pt, :])
                    nc.vector.tensor_tensor(out=sB[pt, :], in0=M0sb[pt, :],
                                            in1=astk[pt, :], op=ALU.mult)
                    nc.vector.tensor_reduce(q0c[pt, :], sB[pt, :],
                                            axis=mybir.AxisListType.X,
                                            op=ALU.add)
                    # ubv = KAP*(q0 - arow) + corr + TCO*t into pack col 0;
                    # colA / colB = arow - colA / t into cols 1-3
                    nc.vector.tensor_tensor(out=pack[pt, 0:1], in0=q0c[pt, :],
                                            in1=arow[pt, :], op=ALU.subtract)
                    nc.vector.scalar_tensor_tensor(
                        out=pack[pt, 0:1], in0=pack[pt, 0:1],
                        scalar=float(KAP), in1=ps_corr[pt, :],
                        op0=ALU.mult, op1=ALU.add)
                    nc.vector.scalar_tensor_tensor(
                        out=pack[pt, 0:1], in0=ps_t[pt, :],
                        scalar=float(TCO), in1=pack[pt, 0:1],
                        op0=ALU.mult, op1=ALU.add)
                    nc.vector.tensor_copy(pack[pt, 3:4], ps_t[pt, :])
                    nc.vector.tensor_tensor(out=pack[pt, 2:3],
                                            in0=arow[pt, :],
                                            in1=colA[pt, :], op=ALU.subtract)
                    nc.vector.tensor_copy(pack[pt, 1:2], colA[pt, :])
                    # transpose the 4 pack columns into the partition-0 row
                    # (ps_m bank 0 is free again: stats above read it first)
                    for k in range(4):
                        nc.tensor.matmul(
                            ps_row[0:1,
                                   128 * k + 64 * i:128 * k + 64 * i + 64],
                            pack[pt, k:k + 1], idm[pt, :],
                            is_transpose=True, start=True, stop=True,
                            tile_position=(64 * i, 0),
                            skip_group_check=True)

                slot_tail(0, pea)

                # ---- swap back to EXP: slot-b K chunks + pair exp ----
                for r in range(6):
                    sl = slice(NM * r, NM * (r + 1))
                    nc.scalar.activation(ktb[:, sl], dist[:, sl], AF.Exp,
                                         scale=lbt, bias=zero)
                    for fs in (slice(0, 512), slice(512, NM)):
                        nc.tensor.matmul(ps_m[64:128, fs],
                                         atp[:, D * r:D * r + 64],
                                         ktb[:, NM * r + fs.start:
                                              NM * r + fs.stop],
                                         start=(r == 0), stop=(r == 5),
                                         tile_position=(0, 64),
                                         skip_group_check=True)
                peb = sml.tile([128, NBLK], BF16, name="peb")
                nc.scalar.activation(peb[:], pdist[:], AF.Exp,
                                     bias=zero, scale=lbt)

                slot_tail(1, peb)

                # ---- partition-0 assembly ----
                frow = sml.tile([1, 512], F32, name="frow")
                nc.vector.tensor_copy(frow[:], ps_row)

                def strided(row, col, *rest):
                    ap = frow[0:1, 128 * row + col:128 * row + col + 1]
                    return bass.AP(ap.tensor, ap.offset,
                                   [ap.ap[0], *rest])

                XXv = strided(1, PPC, [64, 2])
                XY0v = strided(2, PPC, [64, 2])
                YYv = strided(2, PPC + 1, [64, 2])
                sev = strided(3, PPC, [64, 2])
                # s0t = XX + YX + XY0 + YY in one grouped reduce
                quad = strided(1, PPC, [64, 2], [128, 2], [1, 2])
                s0t = sml.tile([1, 2], F32, name="s0t")
                nc.vector.tensor_reduce(s0t[:], quad,
                                        axis=mybir.AxisListType.XY,
                                        op=ALU.add)
                ck = sml.tile([1, 2], F32, name="ck")
                nc.vector.tensor_tensor(out=ck[:], in0=s0t[:], in1=sev,
                                        op=ALU.subtract)
                nc.vector.scalar_tensor_tensor(
                    out=ck[:], in0=ck[:], scalar=float(IC1), in1=aux4i,
                    op0=ALU.mult, op1=ALU.subtract)
                u1 = sml.tile([1, 2], F32, name="u1")
                nc.vector.tensor_tensor(out=u1[:], in0=XXv, in1=YYv,
                                        op=ALU.add)
                nc.vector.scalar_tensor_tensor(
                    out=u1[:], in0=u1[:], scalar=float(IC1), in1=aux4i,
                    op0=ALU.mult, op1=ALU.subtract)
                u2 = sml.tile([1, 2], F32, name="u2")
                nc.vector.tensor_tensor(out=u2[:], in0=XY0v, in1=sev,
                                        op=ALU.subtract)
                uF = sml.tile([1, 2], F32, name="uF")
                nc.vector.scalar_tensor_tensor(
                    out=uF[:], in0=u2[:], scalar=float(-2.0 * IC2), in1=u1[:],
                    op0=ALU.mult, op1=ALU.add)

                ubc = sml.tile([1, 2 * PPC], F32, name="ubc")
                ub0 = frow[0:1, 0:1]
                ub_src = bass.AP(ub0.tensor, ub0.offset,
                                 [ub0.ap[0], [64, 2], [1, PPC]])
                ckap = ck[0:1, 0:2]
                ck_b = bass.AP(ckap.tensor, ckap.offset,
                               [ckap.ap[0], [1, 2], [0, PPC]])
                nc.vector.tensor_tensor(
                    out=ubc[0:1, :].rearrange("o (k p) -> o k p", p=PPC),
                    in0=ub_src, in1=ck_b, op=ALU.add)
                nc.sync.dma_start(
                    out=out_d[:, 0:1],
                    in_=uF[0:1, :].rearrange("o (k w) -> o k w", w=1))
                nc.sync.dma_start(
                    out=out_d[:, 1:1 + PPC],
                    in_=ubc[0:1, :].rearrange("o (k p) -> o k p", p=PPC))

    nc.compile()
    return nc


def _host_prep(X, Y, bandwidths, perms):
    X = np.ascontiguousarray(X, np.float32)
    Y = np.ascontiguousarray(Y, np.float32)
    perms = np.ascontiguousarray(perms, np.int32)
    Zf = np.concatenate([X, Y], 0)
    Zt = Zf.T.astype(np.float32)
    sq = (Zf.astype(np.float64) ** 2).sum(1).astype(np.float32)
    b = np.asarray(bandwidths, np.float64)

    zlr = np.zeros((D + 1, 2 * NM), np.float32)
    zlr[0:D, NM:] = Zt
    zlr[D, NM:] = 1.0
    zlr[0:D, 0:NM] = -2.0 * Zt
    zlr[D, 0:NM] = sq + BIAS

    idm = np.tile(np.eye(64, dtype=np.float32), (2, 1))

    maps = []
    for cid in range(NC):
        ka, kb = (0, 1) if cid < 4 else (2, 3)
        q = cid % 4
        pm = perms[q * PPC:(q + 1) * PPC]

        A = np.zeros((ROWS, NM), np.float32)
        A[np.arange(PPC)[:, None], pm[:, :N]] = 1
        A[PPC, :N] = 1
        A[PPC + 1, N:] = 1
        astk = np.zeros((128, NM), np.float32)
        astk[0:ROWS] = A
        astk[64:64 + ROWS] = A
        atp = np.zeros((128, 6 * D), np.float32)
        for c in range(6):
            atp[:, D * c:D * c + ROWS] = A[:, 128 * c:128 * (c + 1)].T
        A1 = A[:PPC, :N]
        A2 = A[:PPC, N:]
        Wc = (-KAP * (A1 * A2) + CB1 * A1 + CB2 * A2).astype(np.float32)
        wct = np.zeros((128, 3 * D), np.float32)
        for c in range(3):
            wct[:, D * c:D * c + PPC] = Wc[:, 128 * c:128 * (c + 1)].T
        bfp = np.zeros((128, NM + 6 * D + 3 * D), np.float32)
        bfp[:, 0:NM] = astk
        bfp[:, NM:NM + 6 * D] = atp
        bfp[:, NM + 6 * D:NM + 9 * D] = wct

        # pair partial squares: perm p pair j at lane (384p+j)%128, block
        # (384p+j)//128, 4 groups of 16 dims. Stripe pairs (j, 384+j) fill
        # blocks 3*PPC..3*PPC+2; stripe hits inside perm rows get a huge
        # sentinel so exp -> 0 (the zeroed K stripe).
        pX = pm[:, :N].astype(np.int64).ravel()
        pY = pm[:, N:].astype(np.int64).ravel()
        pdv = (Zf[pX] - Zf[pY]).astype(np.float32) ** 2
        psq = pdv.reshape(-1, 4, 16).sum(2) + np.float32(BIAS / 4)
        psq[pY == pX + N] = 1e6
        sdv = (Zf[:N] - Zf[N:]).astype(np.float32) ** 2
        psq = np.concatenate(
            [psq, sdv.reshape(-1, 4, 16).sum(2) + np.float32(BIAS / 4)], 0)
        psq = psq.reshape(NBLK, 128, 4).transpose(1, 0, 2).reshape(128, -1)

        fsp = np.zeros((128, 32), np.float32)
        ga = np.float32(-1.0 / (b[ka] * b[ka]))
        lb = np.float32(-1.0 / b[kb])
        sqcols = sq.reshape(6, 128).T
        fsp[:, 0:6] = sqcols
        fsp[:, 6:12] = ga * sqcols
        fsp[:, 12] = ga
        fsp[:, 13] = lb
        fsp[:, 14] = 0.0
        d0a = np.exp(-BIAS / (b[ka] * b[ka]))
        d0b = np.exp(-np.sqrt(BIAS) / b[kb])
        fsp[0, 16] = np.float32(NM * d0a * IC1)
        fsp[0, 17] = np.float32(NM * d0b * IC1)

        maps.append(dict(zlr=zlr, psq=psq.astype(ml_dtypes.bfloat16),
                         bfp=bfp.astype(ml_dtypes.bfloat16), fsp=fsp,
                         idm=idm))
    return maps


_NC_CACHE = None


def _get_nc():
    global _NC_CACHE
    if _NC_CACHE is None:
        _NC_CACHE = _build()
    return _NC_CACHE


def _merge(results):
    full = np.zeros((4, 1 + NPER), np.float32)
    for cid in range(NC):
        ka, kb = (0, 1) if cid < 4 else (2, 3)
        q = cid % 4
        o = results[cid]["out"]
        full[ka, 1 + q * PPC:1 + (q + 1) * PPC] = o[0, 1:]
        full[kb, 1 + q * PPC:1 + (q + 1) * PPC] = o[1, 1:]
        if q == 0:
            full[ka, 0] = o[0, 0]
            full[kb, 0] = o[1, 0]
    return full


def kernel(X, Y, bandwidths, perms):
    nc = _get_nc()
    in_maps = _host_prep(X, Y, bandwidths, perms)
    res = bass_utils.run_bass_kernel_spmd(nc, in_maps, list(range(NC)))
    return _merge(res.results)


# revision 23
# speedup vs baseline: 1.4047x; 1.0338x over previous
"""Trainium2 Bass kernel for the 4-kernel MMD permutation test (nn_DUAL_78237124264373).

Sharding: 8 cores = 2 kernel-pairs x 4 permutation quarters. Core c<4 computes
kernels (0,1) [gaussian, laplacian] for perms [50*(c%4), 50*(c%4)+50); core
c>=4 the same for kernels (2,3). The host merges the [2, 1+50] per-core
outputs, so each core only ever evaluates TWO kernel matrices and the
activation-table sequence is exactly EXP (pre-warmed) -> SQRT -> EXP: the
swap points are pinned with zero-valued bias/scale tokens that data-depend
on the previous block's last op, so the Tile scheduler cannot interleave.

Per-core pipeline (slot a = gaussian, slot b = laplacian):
  d2 = L^T R on the PE in f32r (L = [Zt; 1], R = [-2 Zt; sq + B]), 12 PSUM
  pieces (6 row tiles x 512+256) in a 4-deep PSUM pool. Slot-a K = exp(ga*d2)
  straight out of PSUM with its M0 = A_aug K chunk matmul right behind, while
  the DVE lands clamped d2 in SBUF for the sqrt block. After the swap back to
  EXP, slot-b K = exp(lb*dist) runs chunk-by-chunk with M0 trailing. Each
  slot's row stats (aKa, aK1, colA), U_b vector and PE transpose into the
  partition-0 assembly row run as soon as that slot's M0 completes, so slot
  a's tail hides under the SQRT/slot-b window. U_b = KAP*(aKa - aK1) +
  W_corr @ e + (2/c2)*t + ck, with pair sums t reduced from host-gathered
  16-wide partial squares and e the K0[j, 384+j] stripe (3 extra pair
  blocks). The final scalar assembly (U, ck) happens on partition 0.
"""

import sys

import numpy as np

if "/opt/trn_rl_repo" not in sys.path:
    sys.path.insert(0, "/opt/trn_rl_repo")

import ml_dtypes

import concourse.bacc as bacc
import concourse.bass as bass
import concourse.mybir as mybir
import concourse.tile as tile
from concourse import bass_utils

N = 384
NM = 768
D = 64
NPER = 200
NC = 8
PPC = 50                      # perms per core
ROWS = PPC + 2                # + X-identity + Y-identity rows
NBLK = 3 * PPC + 3            # pair blocks of 128: 50 perms x 3 + stripe x 3
BIAS = 1e-3                   # keeps d2 > 0 under f32r rounding (see d0c)
C1 = float(N * (N - 1))
C2 = float(N * N)
KAP = np.float32(2.0 / C1 + 2.0 / C2)
CB1 = np.float32(1.0 / C1 + 2.0 / C2)
CB2 = np.float32(1.0 / C1)
TCO = np.float32(2.0 / C2)
IC1 = np.float32(1.0 / C1)
IC2 = np.float32(1.0 / C2)

F32 = mybir.dt.float32
F32R = mybir.dt.float32r
BF16 = mybir.dt.bfloat16
AF = mybir.ActivationFunctionType
ALU = mybir.AluOpType


def _build():
    nc = bacc.Bacc("TRN2", target_bir_lowering=False, debug=False)
    with tile.TileContext(nc) as tc:
        with tc.tile_pool(name="dram", bufs=1, space="DRAM") as dram, \
             tc.tile_pool(name="io", bufs=1) as io, \
             tc.tile_pool(name="big", bufs=1) as big, \
             tc.tile_pool(name="scr", bufs=1) as scr, \
             tc.tile_pool(name="sml", bufs=1) as sml:

            def din(name, shape, dt=F32):
                return dram.tile(shape, dt, kind="ExternalInput", name=name,
                                 uniquify=False)

            zlr_d = din("zlr", [D + 1, 2 * NM], F32R)
            psq_d = din("psq", [128, NBLK * 4], BF16)
            bfp_d = din("bfp", [128, NM + 6 * D + 3 * D], BF16)
            fsp_d = din("fsp", [128, 32], F32)
            idm_d = din("idm", [128, 64], F32)
            out_d = dram.tile([2, 1 + PPC], F32, kind="ExternalOutput",
                              name="out", uniquify=False)

            # ---- input DMAs; zlr is [R | L] so the first piece lands first
            zlr = io.tile([D + 1, 2 * NM], F32R, name="zlr_sb")
            nc.sync.dma_start(out=zlr[:, 0:896], in_=zlr_d[:, 0:896])
            nc.sync.dma_start(out=zlr[:, 896:], in_=zlr_d[:, 896:])
            psq = io.tile([128, NBLK * 4], BF16, name="psq_sb")
            nc.sync.dma_start(out=psq[:], in_=psq_d[:])
            bfp = io.tile([128, NM + 6 * D + 3 * D], BF16, name="bfp_sb")
            nc.sync.dma_start(out=bfp[:], in_=bfp_d[:])
            fsp = io.tile([128, 32], F32, name="fsp_sb")
            nc.sync.dma_start(out=fsp[:], in_=fsp_d[:])
            idm = io.tile([128, 64], F32, name="idm_sb")
            nc.sync.dma_start(out=idm[:], in_=idm_d[:])

            zr = zlr[:, 0:NM]
            zl = zlr[:, NM:2 * NM]
            astk = bfp[:, 0:NM]                      # A rows at 0-51 / 64-115
            atp = bfp[:, NM:NM + 6 * D]              # A^T chunks, 64-padded
            wct = bfp[:, NM + 6 * D:NM + 9 * D]      # W_corr^T chunks
            sqc = fsp[:, 0:6]                        # sq columns per row tile
            gbias = fsp[:, 6:12]                     # ga * sq per row tile
            ga = fsp[:, 12:13]
            lb = fsp[:, 13:14]
            zero = fsp[:, 14:15]
            aux4i = fsp[0:1, 16:18]                  # 768*d0c*IC1 per slot

            ones = io.tile([128, 1], F32, name="ones_sb")
            nc.vector.memset(ones[:], 1.0)
            onesb = io.tile([128, 1], BF16, name="onesb_sb")
            nc.vector.memset(onesb[:], 1.0)

            d2sb = big.tile([128, 6 * NM], F32, name="d2sb")
            dist = big.tile([128, 6 * NM], F32, name="dist_sb")
            kta = big.tile([128, 6 * NM], BF16, name="kta")
            ktb = big.tile([128, 6 * NM], BF16, name="ktb")
            M0sb = big.tile([128, NM], F32, name="M0sb")
            sA = scr.tile([128, N], F32, name="sA")
            sB = scr.tile([128, NM], F32, name="sB")
            pair2 = sml.tile([128, NBLK], F32, name="pair2")
            arow = sml.tile([128, 1], F32, name="arow")
            colA = sml.tile([128, 1], F32, name="colA")
            q0c = sml.tile([128, 1], F32, name="q0c")
            pack = sml.tile([128, 4], F32, name="pack")

            # warm the EXP activation table while DMAs are in flight
            warm = sml.tile([128, 1], F32, name="warm")
            nc.scalar.activation(warm[0:1, :], ones[0:1, :], AF.Exp,
                                 bias=0.0, scale=1.0)

            with tc.tile_pool(name="psA", bufs=4, space="PSUM") as psA, \
                 tc.tile_pool(name="psB", bufs=1, space="PSUM") as psB, \
                 tc.tile_pool(name="psC", bufs=1, space="PSUM") as psC:

                ps_m = psB.tile([128, NM], F32, name="ps_m")
                ps_t = psC.tile([128, 1], F32, name="ps_t")
                ps_corr = psC.tile([128, 1], F32, name="ps_corr")
                ps_row = ps_m[0:1, 0:512]

                # ---- pair d2: reduce the host 16-wide partial squares ----
                psq3 = psq.rearrange("p (b d) -> p b d", d=4)
                nc.vector.tensor_reduce(pair2[:], psq3[:],
                                        axis=mybir.AxisListType.X, op=ALU.add)

                # ---- d2 phase: f32r matmuls in 512-col PSUM pieces; slot-a
                # exp + M0 chunk from PSUM; DVE lands d2 in SBUF for sqrt ----
                for r in range(6):
                    lhs = zl[:, 128 * r:128 * (r + 1)]
                    for fs in (slice(0, 512), slice(512, NM)):
                        w = fs.stop - fs.start
                        ps_d2 = psA.tile([128, 512], F32, tag="d2",
                                         name=f"ps_d2_{r}_{fs.start}")
                        nc.tensor.matmul(ps_d2[:, 0:w], lhs, zr[:, fs],
                                         start=True, stop=True)
                        sl = slice(NM * r + fs.start, NM * r + fs.stop)
                        nc.scalar.activation(kta[:, sl], ps_d2[:, 0:w],
                                             AF.Exp, scale=ga,
                                             bias=gbias[:, r:r + 1])
                        nc.vector.tensor_scalar(
                            out=d2sb[:, sl], in0=ps_d2[:, 0:w],
                            scalar1=sqc[:, r:r + 1], scalar2=0.0,
                            op0=ALU.add, op1=ALU.max)

                # slot-a M0 runs after the whole d2 phase: interleaving it
                # would chain each d2 matmul behind the previous exp in the
                # PE's in-order queue (it fills the idle PE sqrt window)
                for r in range(6):
                    for fs in (slice(0, 512), slice(512, NM)):
                        nc.tensor.matmul(ps_m[0:64, fs],
                                         atp[:, D * r:D * r + 64],
                                         kta[:, NM * r + fs.start:
                                              NM * r + fs.stop],
                                         start=(r == 0), stop=(r == 5),
                                         tile_position=(0, 0),
                                         skip_group_check=True)

                # slot-a pair exp rides the warm EXP table before the swap
                pea = sml.tile([128, NBLK], BF16, name="pea")
                nc.scalar.activation(pea[:], pair2[:], AF.Exp,
                                     bias=zero, scale=ga)
                # zb: zero bias that depends on the last EXP-block op, so
                # the scheduler cannot move the SQRT block earlier
                zb1 = sml.tile([128, 1], F32, name="zb1")
                nc.vector.tensor_scalar(
                    out=zb1[:], in0=kta[:, 6 * NM - 1:6 * NM],
                    scalar1=0.0, scalar2=0.0, op0=ALU.mult, op1=ALU.add)
                zb = sml.tile([128, 1], F32, name="zb")
                nc.vector.tensor_tensor(out=zb[:], in0=zb1[:],
                                        in1=pea[:, 0:1], op=ALU.mult)

                # ---- swap to SQRT: dist halves, then pair dist ----
                for h in range(2):
                    hs = slice(3 * NM * h, 3 * NM * (h + 1))
                    nc.scalar.activation(dist[:, hs], d2sb[:, hs], AF.Sqrt,
                                         bias=zb, scale=1.0)
                zb2 = sml.tile([128, 1], F32, name="zb2")
                nc.vector.tensor_scalar(
                    out=zb2[:], in0=dist[:, 6 * NM - 1:6 * NM],
                    scalar1=0.0, scalar2=0.0, op0=ALU.mult, op1=ALU.add)
                pdist = sml.tile([128, NBLK], F32, name="pdist")
                nc.scalar.activation(pdist[:], pair2[:], AF.Sqrt,
                                     bias=zb2, scale=1.0)
                # lbt == lb, but depends on the last SQRT-block op
                lbt = sml.tile([128, 1], F32, name="lbt")
                nc.vector.tensor_scalar(
                    out=lbt[:], in0=pdist[:, 0:1], scalar1=0.0,
                    scalar2=lb, op0=ALU.mult, op1=ALU.add)

                def slot_tail(i, pe):
                    """Pair sums, corrections, row stats, ubv and the PE
                    transpose for slot i; runs as soon as its M0 stops."""
                    pt = slice(64 * i, 64 * i + 64)
                    # t3: per-perm 3-block partial sums; group PPC holds the
                    # stripe so t[50] = sum(e) lands in ps_t for free
                    pe3 = pe.rearrange("p (g t) -> p g t", t=3)
                    t3 = sml.tile([128, 64], BF16, name=f"t3_{i}")
                    nc.vector.memset(t3[:, PPC + 1:64], 0.0)
                    with nc.allow_low_precision(reason="3-wide bf16 sum"):
                        nc.vector.tensor_reduce(t3[:, 0:PPC + 1], pe3[:],
                                                axis=mybir.AxisListType.X,
                                                op=ALU.add)
                    nc.tensor.matmul(ps_t[pt, :], t3[:], onesb[:],
                                     start=True, stop=True,
                                     tile_position=(0, 64 * i),
                                     skip_group_check=True)
                    for c in range(3):
                        nc.tensor.matmul(
                            ps_corr[pt, :], wct[:, D * c:D * c + 64],
                            pe[:, 3 * PPC + c:3 * PPC + c + 1],
                            start=(c == 0), stop=(c == 2),
                            tile_position=(0, 64 * i),
                            skip_group_check=True)
                    # row stats off this slot's half of ps_m
                    nc.vector.tensor_scalar(
                        out=M0sb[pt, :], in0=ps_m[pt, :], scalar1=1.0,
                        scalar2=0.0, op0=ALU.mult, op1=ALU.add,
                        accum_out=arow[pt, :])
                    nc.vector.tensor_scalar(
                        out=sA[pt, :], in0=M0sb[pt, 0:N], scalar1=1.0,
                        scalar2=0.0, op0=ALU.mult, op1=ALU.add,
                        accum_out=colA[pt, :])
                    nc.vector.scalar_tensor_tensor(
                        out=sB[pt, :], in0=M0sb[pt, :], scalar=1.0,
                        in1=astk[pt, :], op0=ALU.mult, op1=ALU.mult,
                        accum_out=q0c[pt, :])
                    # ubv = KAP*(q0 - arow) + corr + TCO*t into pack col 0;
                    # colA / colB = arow - colA / t into cols 1-3
                    nc.vector.tensor_tensor(out=pack[pt, 0:1], in0=q0c[pt, :],
                                            in1=arow[pt, :], op=ALU.subtract)
                    nc.vector.scalar_tensor_tensor(
                        out=pack[pt, 0:1], in0=pack[pt, 0:1],
                        scalar=float(KAP), in1=ps_corr[pt, :],
                        op0=ALU.mult, op1=ALU.add)
                    nc.vector.scalar_tensor_tensor(
                        out=pack[pt, 0:1], in0=ps_t[pt, :],
                        scalar=float(TCO), in1=pack[pt, 0:1],
                        op0=ALU.mult, op1=ALU.add)
                    nc.vector.tensor_copy(pack[pt, 3:4], ps_t[pt, :])
                    nc.vector.tensor_tensor(out=pack[pt, 2:3],
                                            in0=arow[pt, :],
                                            in1=colA[pt, :], op=ALU.subtract)
                    nc.vector.tensor_copy(pack[pt, 1:2], colA[pt, :])
                    # transpose the 4 pack columns into the partition-0 row
                    # (ps_m bank 0 is free again: stats above read it first)
                    for k in range(4):
                        nc.tensor.matmul(
                            ps_row[0:1,
                                   128 * k + 64 * i:128 * k + 64 * i + 64],
                            pack[pt, k:k + 1], idm[pt, :],
                            is_transpose=True, start=True, stop=True,
                            tile_position=(64 * i, 0),
                            skip_group_check=True)

                slot_tail(0, pea)

                # ---- swap back to EXP: slot-b K chunks + pair exp ----
                for r in range(6):
                    sl = slice(NM * r, NM * (r + 1))
                    nc.scalar.activation(ktb[:, sl], dist[:, sl], AF.Exp,
                                         scale=lbt, bias=zero)
                    for fs in (slice(0, 512), slice(512, NM)):
                        nc.tensor.matmul(ps_m[64:128, fs],
                                         atp[:, D * r:D * r + 64],
                                         ktb[:, NM * r + fs.start:
                                              NM * r + fs.stop],
                                         start=(r == 0), stop=(r == 5),
                                         tile_position=(0, 64),
                                         skip_group_check=True)
                peb = sml.tile([128, NBLK], BF16, name="peb")
                nc.scalar.activation(peb[:], pdist[:], AF.Exp,
                                     bias=zero, scale=lbt)

                slot_tail(1, peb)

                # ---- partition-0 assembly ----
                frow = sml.tile([1, 512], F32, name="frow")
                nc.vector.tensor_copy(frow[:], ps_row)

                def strided(row, col, *rest):
                    ap = frow[0:1, 128 * row + col:128 * row + col + 1]
                    return bass.AP(ap.tensor, ap.offset,
                                   [ap.ap[0], *rest])

                XXv = strided(1, PPC, [64, 2])
                XY0v = strided(2, PPC, [64, 2])
                YYv = strided(2, PPC + 1, [64, 2])
                sev = strided(3, PPC, [64, 2])
                # s0t = XX + YX + XY0 + YY in one grouped reduce
                quad = strided(1, PPC, [64, 2], [128, 2], [1, 2])
                s0t = sml.tile([1, 2], F32, name="s0t")
                nc.vector.tensor_reduce(s0t[:], quad,
                                        axis=mybir.AxisListType.XY,
                                        op=ALU.add)
                ck = sml.tile([1, 2], F32, name="ck")
                nc.vector.tensor_tensor(out=ck[:], in0=s0t[:], in1=sev,
                                        op=ALU.subtract)
                nc.vector.scalar_tensor_tensor(
                    out=ck[:], in0=ck[:], scalar=float(IC1), in1=aux4i,
                    op0=ALU.mult, op1=ALU.subtract)
                u1 = sml.tile([1, 2], F32, name="u1")
                nc.vector.tensor_tensor(out=u1[:], in0=XXv, in1=YYv,
                                        op=ALU.add)
                nc.vector.scalar_tensor_tensor(
                    out=u1[:], in0=u1[:], scalar=float(IC1), in1=aux4i,
                    op0=ALU.mult, op1=ALU.subtract)
                u2 = sml.tile([1, 2], F32, name="u2")
                nc.vector.tensor_tensor(out=u2[:], in0=XY0v, in1=sev,
                                        op=ALU.subtract)
                uF = sml.tile([1, 2], F32, name="uF")
                nc.vector.scalar_tensor_tensor(
                    out=uF[:], in0=u2[:], scalar=float(-2.0 * IC2), in1=u1[:],
                    op0=ALU.mult, op1=ALU.add)

                ubc = sml.tile([1, 2 * PPC], F32, name="ubc")
                ub0 = frow[0:1, 0:1]
                ub_src = bass.AP(ub0.tensor, ub0.offset,
                                 [ub0.ap[0], [64, 2], [1, PPC]])
                ckap = ck[0:1, 0:2]
                ck_b = bass.AP(ckap.tensor, ckap.offset,
                               [ckap.ap[0], [1, 2], [0, PPC]])
                nc.vector.tensor_tensor(
                    out=ubc[0:1, :].rearrange("o (k p) -> o k p", p=PPC),
                    in0=ub_src, in1=ck_b, op=ALU.add)
                nc.sync.dma_start(
                    out=out_d[:, 0:1],
                    in_=uF[0:1, :].rearrange("o (k w) -> o k w", w=1))
                nc.sync.dma_start(
                    out=out_d[:, 1:1 + PPC],
                    in_=ubc[0:1, :].rearrange("o (k p) -> o k p", p=PPC))

    nc.compile()
    return nc


def _host_prep(X, Y, bandwidths, perms):
    X = np.ascontiguousarray(X, np.float32)
    Y = np.ascontiguousarray(Y, np.float32)
    perms = np.ascontiguousarray(perms, np.int32)
    Zf = np.concatenate([X, Y], 0)
    Zt = Zf.T.astype(np.float32)
    sq = (Zf.astype(np.float64) ** 2).sum(1).astype(np.float32)
    b = np.asarray(bandwidths, np.float64)

    zlr = np.zeros((D + 1, 2 * NM), np.float32)
    zlr[0:D, NM:] = Zt
    zlr[D, NM:] = 1.0
    zlr[0:D, 0:NM] = -2.0 * Zt
    zlr[D, 0:NM] = sq + BIAS

    idm = np.tile(np.eye(64, dtype=np.float32), (2, 1))

    maps = []
    for cid in range(NC):
        ka, kb = (0, 1) if cid < 4 else (2, 3)
        q = cid % 4
        pm = perms[q * PPC:(q + 1) * PPC]

        A = np.zeros((ROWS, NM), np.float32)
        A[np.arange(PPC)[:, None], pm[:, :N]] = 1
        A[PPC, :N] = 1
        A[PPC + 1, N:] = 1
        astk = np.zeros((128, NM), np.float32)
        astk[0:ROWS] = A
        astk[64:64 + ROWS] = A
        atp = np.zeros((128, 6 * D), np.float32)
        for c in range(6):
            atp[:, D * c:D * c + ROWS] = A[:, 128 * c:128 * (c + 1)].T
        A1 = A[:PPC, :N]
        A2 = A[:PPC, N:]
        Wc = (-KAP * (A1 * A2) + CB1 * A1 + CB2 * A2).astype(np.float32)
        wct = np.zeros((128, 3 * D), np.float32)
        for c in range(3):
            wct[:, D * c:D * c + PPC] = Wc[:, 128 * c:128 * (c + 1)].T
        bfp = np.zeros((128, NM + 6 * D + 3 * D), np.float32)
        bfp[:, 0:NM] = astk
        bfp[:, NM:NM + 6 * D] = atp
        bfp[:, NM + 6 * D:NM + 9 * D] = wct

        # pair partial squares: perm p pair j at lane (384p+j)%128, block
        # (384p+j)//128, 4 groups of 16 dims. Stripe pairs (j, 384+j) fill
        # blocks 3*PPC..3*PPC+2; stripe hits inside perm rows get a huge
        # sentinel so exp -> 0 (the zeroed K stripe).
        pX = pm[:, :N].astype(np.int64).ravel()
        pY = pm[:, N:].astype(np.int64).ravel()
        pdv = (Zf[pX] - Zf[pY]).astype(np.float32) ** 2
        psq = pdv.reshape(-1, 4, 16).sum(2) + np.float32(BIAS / 4)
        psq[pY == pX + N] = 1e6
        sdv = (Zf[:N] - Zf[N:]).astype(np.float32) ** 2
        psq = np.concatenate(
            [psq, sdv.reshape(-1, 4, 16).sum(2) + np.float32(BIAS / 4)], 0)
        psq = psq.reshape(NBLK, 128, 4).transpose(1, 0, 2).reshape(128, -1)

        fsp = np.zeros((128, 32), np.float32)
        ga = np.float32(-1.0 / (b[ka] * b[ka]))
        lb = np.float32(-1.0 / b[kb])
        sqcols = sq.reshape(6, 128).T
        fsp[:, 0:6] = sqcols
        fsp[:, 6:12] = ga * sqcols
        fsp[:, 12] = ga
        fsp[:, 13] = lb
        fsp[:, 14] = 0.0
        d0a = np.exp(-BIAS / (b[ka] * b[ka]))
        d0b = np.exp(-np.sqrt(BIAS) / b[kb])
        fsp[0, 16] = np.float32(NM * d0a * IC1)
        fsp[0, 17] = np.float32(NM * d0b * IC1)

        maps.append(dict(zlr=zlr, psq=psq.astype(ml_dtypes.bfloat16),
                         bfp=bfp.astype(ml_dtypes.bfloat16), fsp=fsp,
                         idm=idm))
    return maps


_NC_CACHE = None


def _get_nc():
    global _NC_CACHE
    if _NC_CACHE is None:
        _NC_CACHE = _build()
    return _NC_CACHE


def _merge(results):
    full = np.zeros((4, 1 + NPER), np.float32)
    for cid in range(NC):
        ka, kb = (0, 1) if cid < 4 else (2, 3)
        q = cid % 4
        o = results[cid]["out"]
        full[ka, 1 + q * PPC:1 + (q + 1) * PPC] = o[0, 1:]
        full[kb, 1 + q * PPC:1 + (q + 1) * PPC] = o[1, 1:]
        if q == 0:
            full[ka, 0] = o[0, 0]
            full[kb, 0] = o[1, 0]
    return full


def kernel(X, Y, bandwidths, perms):
    nc = _get_nc()
    in_maps = _host_prep(X, Y, bandwidths, perms)
    res = bass_utils.run_bass_kernel_spmd(nc, in_maps, list(range(NC)))
    return _merge(res.results)
